# revision 1
# baseline (speedup 1.0000x reference)
"""Trainium2 Bass kernel for nn_Axon_53489522704543 (scatter_memory).

Computation (reference):
    att = clip(attenuation, 0, 1); decay = 0.9**delays
    signals[b,s,br] = spikes[b,s] * att[s,br] * decay[s,br]
    out[b,t] = sum over (s,br) with target_indices[s,br]==t of signals[b,s,br]

Strategy: source-parallel over 8 cores (2048 sources each). On each core,
the scatter is computed exactly with TensorE: for each tile of 128
(source, branch) pairs we build one-hot matrices of the target's high/low
7 bits and contract pairs on the PE:

    psum[hi, (b, lo)] += OH_hi[i, hi].T @ (v[i, b] * OH_lo[i, lo])

accumulating the full [128 hi, 32 b * 128 lo] partial output in PSUM over
all 1024 tiles. One-hots are exact in fp16; v = W*spike is rounded to
fp16 (PSUM accumulates fp32). Host sums the 8 per-core partials.
"""

import numpy as np

import concourse.bacc as bacc
import concourse.bass as bass
import concourse.mybir as mybir
import concourse.tile as tile
from concourse.alu_op_type import AluOpType
from concourse.bass_utils import run_bass_kernel_spmd

N_CORES = 8
S = 16384          # sources
T = 16384          # targets
BR = 64            # branches
B = 32             # batch
SC = S // N_CORES  # sources per core (2048)
NBLK = SC // 128   # source tiles per core (16)
NTILE = NBLK * BR  # pair tiles per core (1024)
SMOOTHING = 0.9

F32 = mybir.dt.float32
F16 = mybir.dt.float16

_CACHE = {}
REPEAT = 1  # >1: wrap the compute loop in For_i for timing measurements


def _build():
    nc = bacc.Bacc("TRN2", target_bir_lowering=False, debug=False,
                   num_devices=N_CORES)

    spk_d = nc.dram_tensor("spk", [SC, B], F16, kind="ExternalInput")
    att_d = nc.dram_tensor("att", [SC, BR], F32, kind="ExternalInput")
    dly_d = nc.dram_tensor("dly", [SC, BR], F32, kind="ExternalInput")
    hi_d = nc.dram_tensor("hi", [SC, BR], F32, kind="ExternalInput")
    lo_d = nc.dram_tensor("lo", [SC, BR], F32, kind="ExternalInput")
    iot_d = nc.dram_tensor("iot", [128, 128], F16, kind="ExternalInput")
    iotr_d = nc.dram_tensor("iotr", [128, B * 128], F16, kind="ExternalInput")
    part_d = nc.dram_tensor("part", [128, B * 128], F32, kind="ExternalOutput")

    with tile.TileContext(nc) as tc:
        with (
            tc.tile_pool(name="slab", bufs=1) as slab,
            tc.tile_pool(name="oh", bufs=3) as ohp,
            tc.tile_pool(name="rhsp", bufs=2) as rhsp,
            tc.tile_pool(name="psum", bufs=1, space="PSUM") as psp,
        ):
            # resident slabs: [128, NBLK*BR] layout, col = blk*BR + br,
            # partition p = source blk*128 + p
            def slab_ap(dram):
                return bass.AP(dram, 0, [[BR, 128], [128 * BR, NBLK], [1, BR]])

            att_t = slab.tile([128, NBLK * BR], F32, tag="att")
            dly_t = slab.tile([128, NBLK * BR], F32, tag="dly")
            hi_t = slab.tile([128, NBLK * BR], F32, tag="hi")
            lo_t = slab.tile([128, NBLK * BR], F32, tag="lo")
            w_t = slab.tile([128, NBLK * BR], F32, tag="w")
            iot_t = slab.tile([128, 128], F16, tag="iot")
            iotr_t = slab.tile([128, B * 128], F16, tag="iotr")
            spk_t = slab.tile([128, NBLK * B], F16, tag="spk")
            outs_t = slab.tile([128, B * 128], F32, tag="outs")

            nc.sync.dma_start(att_t[:], slab_ap(att_d))
            nc.sync.dma_start(dly_t[:], slab_ap(dly_d))
            nc.sync.dma_start(hi_t[:], slab_ap(hi_d))
            nc.sync.dma_start(lo_t[:], slab_ap(lo_d))
            nc.sync.dma_start(iot_t[:], iot_d.ap())
            nc.sync.dma_start(iotr_t[:], iotr_d.ap())
            nc.sync.dma_start(
                spk_t[:], bass.AP(spk_d, 0, [[B, 128], [128 * B, NBLK], [1, B]]))

            # W = clip(att,0,1) * 0.9^dly, decay via exact 6-term one-hot sum
            nc.vector.tensor_scalar(w_t[:], att_t[:], 0.0, 1.0,
                                    AluOpType.max, AluOpType.min)
            dec_t = slab.tile([128, NBLK * BR], F32, tag="dec")
            trm_t = slab.tile([128, NBLK * BR], F32, tag="trm")
            for k in range(6):
                dst = dec_t if k == 0 else trm_t
                nc.vector.tensor_scalar(dst[:], dly_t[:], float(k),
                                        float(SMOOTHING ** k),
                                        AluOpType.is_equal, AluOpType.mult)
                if k > 0:
                    nc.vector.tensor_tensor(dec_t[:], dec_t[:], trm_t[:],
                                            AluOpType.add)
            nc.vector.tensor_tensor(w_t[:], w_t[:], dec_t[:], AluOpType.mult)

            ps = psp.tile([128, B * 128], F32)

            import contextlib
            rep_ctx = (tc.For_i(0, REPEAT, 1) if REPEAT > 1
                       else contextlib.nullcontext())
            with rep_ctx:
                self_loop_body(nc, tc, ohp, rhsp, ps, iot_t, iotr_t, hi_t, lo_t,
                               w_t, spk_t)

            nc.vector.tensor_copy(outs_t[:], ps[:])
            nc.sync.dma_start(part_d.ap(), outs_t[:])

    nc.compile()
    return nc


def self_loop_body(nc, tc, ohp, rhsp, ps, iot_t, iotr_t, hi_t, lo_t, w_t,
                   spk_t):
    G = 4  # branch-tiles batched per tensor_tensor (share the spike tile)
    W4 = G * B * 128
    if True:
            for blk in range(NBLK):
                for brg in range(BR // G):
                    rhsA4 = rhsp.tile([128, W4], F16, tag="rhsA4")
                    rhs4 = rhsp.tile([128, W4], F16, tag="rhs4")
                    ohs = []
                    for j in range(G):
                        br = brg * G + j
                        col = blk * BR + br
                        oh_hi = ohp.tile([128, 128], F16, tag=f"oh_hi{j}")
                        ohs.append(oh_hi)
                        nc.vector.tensor_scalar(
                            oh_hi[:], iot_t[:], hi_t[:, col:col + 1], None,
                            AluOpType.is_equal)
                        # rhsA4 slice j: [lo == lo_i] * W_i  (4x tensor_scalar)
                        nc.vector.tensor_scalar(
                            rhsA4[:, j * B * 128:(j + 1) * B * 128],
                            iotr_t[:], lo_t[:, col:col + 1],
                            w_t[:, col:col + 1], AluOpType.is_equal,
                            AluOpType.mult)
                    # rhs4 = rhsA4 * spk[i, b] for all G tiles (2x packed tt)
                    _sap = spk_t[:]
                    in1 = bass.AP(_sap.tensor, blk * B,
                                  [[NBLK * B, 128], [0, G * 128], [1, B]])
                    nc.vector.tensor_tensor(rhs4[:], rhsA4[:], in1,
                                            AluOpType.mult)

                    for j in range(G):
                        first = (blk == 0 and brg == 0 and j == 0)
                        last = (blk == NBLK - 1 and brg == BR // G - 1
                                and j == G - 1)
                        for k in range(8):
                            nc.tensor.matmul(
                                ps[:, k * 512:(k + 1) * 512],
                                ohs[j][:],
                                rhs4[:, j * B * 128 + k * 512:
                                     j * B * 128 + (k + 1) * 512],
                                start=first, stop=last)


def kernel(spikes, attenuation, target_indices, delays):
    spikes = np.asarray(spikes, dtype=np.float32)
    attenuation = np.asarray(attenuation, dtype=np.float32)
    tgt = np.asarray(target_indices).astype(np.int64)
    delays_f = np.asarray(delays).astype(np.float32)

    if "nc" not in _CACHE:
        _CACHE["nc"] = _build()
    nc = _CACHE["nc"]

    spikesT = np.ascontiguousarray(spikes.T)              # [S, B]
    hi = (tgt >> 7).astype(np.float32)
    lo = (tgt & 127).astype(np.float32)
    iota = np.broadcast_to(np.arange(128, dtype=np.float16), (128, 128)).copy()
    iotr = np.broadcast_to(np.repeat(np.arange(128), B).astype(np.float16),
                           (128, B * 128)).copy()

    in_maps = []
    for c in range(N_CORES):
        sl = slice(c * SC, (c + 1) * SC)
        in_maps.append({
            "spk": np.ascontiguousarray(spikesT[sl]).astype(np.float16),
            "att": np.ascontiguousarray(attenuation[sl]),
            "dly": np.ascontiguousarray(delays_f[sl]),
            "hi": np.ascontiguousarray(hi[sl]),
            "lo": np.ascontiguousarray(lo[sl]),
            "iot": iota,
            "iotr": iotr,
        })

    res = run_bass_kernel_spmd(nc, in_maps, core_ids=list(range(N_CORES)))
    _CACHE["last_result"] = res

    # part[hi, lo*32 + b] -> out[b, hi*128 + lo]
    acc = np.zeros((128, B * 128), dtype=np.float64)
    for c in range(N_CORES):
        acc += res.results[c]["part"].astype(np.float64)
    out = acc.reshape(128, 128, B).transpose(2, 0, 1).reshape(B, T)
    return out.astype(np.float32)

